# revision 6
# baseline (speedup 1.0000x reference)
"""Trainium2 Bass kernel for the PINN PDE-residual net (nn_Net_PDE).

Computes, for each point p=(x,y,t) of xyt (131072,3):
    h = MLP(p)   (3 -> 256 x6 tanh -> 1)
    res = MU*h_t - K*(h*(h_xx+h_yy) + h_x^2 + h_y^2) - f(p)
    f = sin(pi x) sin(pi y) exp(-t)

Strategy: pure data-parallel over 8 NeuronCores (16384 points each).
Derivatives are propagated forward-mode through the MLP as 6 jet
streams (value, d/dx, d/dy, d/dt, d2/dx2, d2/dy2).  Layout keeps the
MLP width on SBUF partitions (2 halves of 128) and points along the
free dimension, so every layer is plain stationary-weight matmuls plus
elementwise tanh'/tanh'' combinations:

    a   = W^T z          z' = tanh(a)        t1 = 1 - z'^2
    da  = W^T dz         dz' = t1 * da       t2 = -2 z' t1
    d2a = W^T d2z        d2z' = t1*d2a + t2*da^2

The second-derivative stream is kept as the unsummed pair
(u1 = t1*d2a, u2 = t2*da^2); the next layer's matmul accumulates both
into one PSUM tile, saving a vector add.  The final 256->1 projection
uses a (128,8) stacked weight so h, hx, hy, ht and (hxx+hyy) land on
separate partitions of a single PSUM tile; the residual itself is done
in a points-packed (128,128) layout after a DRAM-bounce reshape.
"""

import numpy as np
from contextlib import ExitStack

import concourse.bass as bass
import concourse.bacc as bacc
import concourse.tile as tile
import concourse.mybir as mybir
from concourse.bass_utils import run_bass_kernel_spmd

AF = mybir.ActivationFunctionType
OP = mybir.AluOpType
F32 = mybir.dt.float32
F32R = mybir.dt.float32r

import os as _os

NCORES = 8
NPTS = 131072
S = int(_os.environ.get("BASS_PDE_S", NPTS // NCORES))  # points per core
BLK = 512                   # points per block (one PSUM bank of fp32)
NBLK = S // BLK             # 32
PK = S // 128               # packed free dim = 128
W = 256                     # MLP width
DEPTH = 6                   # hidden (tanh) layers
MU = 1.0
KC = 0.5
PI = float(np.pi)

STREAMS = ["z", "gx", "gy", "gt", "u1x", "u2x", "u1y", "u2y"]


def _r(ap):
    """View an fp32 AP as float32r for full-rate PE matmuls."""
    return ap.bitcast(F32R)


def build_nc():
    nc = bacc.Bacc()

    # ---------------- DRAM I/O ----------------
    xyt_t = nc.dram_tensor("xyt_t", [3, S], F32R, kind="ExternalInput")
    xyt_pack = nc.dram_tensor("xyt_pack", [3, 128, PK], F32, kind="ExternalInput")
    w0_d = nc.dram_tensor("w0", [3, W], F32R, kind="ExternalInput")
    w0r_d = nc.dram_tensor("w0r", [128, 2, 3], F32, kind="ExternalInput")
    w0q_d = nc.dram_tensor("w0q", [128, 2, 2], F32, kind="ExternalInput")
    w_d = {
        l: nc.dram_tensor(f"w{l}", [W, W], F32R, kind="ExternalInput")
        for l in range(1, DEPTH)
    }
    bc_d = {
        l: nc.dram_tensor(f"b{l}c", [128, 2], F32, kind="ExternalInput")
        for l in range(DEPTH)
    }
    w6s_d = nc.dram_tensor("w6s", [128, 2, 5, 8], F32R, kind="ExternalInput")
    b6b_d = nc.dram_tensor("b6b", [128, 1], F32, kind="ExternalInput")
    res_d = nc.dram_tensor("res", [128, PK], F32, kind="ExternalOutput")

    with tile.TileContext(nc) as tc, ExitStack() as ctx:
        const = ctx.enter_context(tc.tile_pool(name="const", bufs=1))
        sb = ctx.enter_context(tc.tile_pool(name="sb", bufs=2))
        tmp = ctx.enter_context(tc.tile_pool(name="tmp", bufs=2))
        xin = ctx.enter_context(tc.tile_pool(name="xin", bufs=3))
        ps = ctx.enter_context(tc.tile_pool(name="ps", bufs=3, space="PSUM"))
        psf = ctx.enter_context(tc.tile_pool(name="psf", bufs=2, space="PSUM"))
        rp = ctx.enter_context(tc.tile_pool(name="rp", bufs=1))
        dram = ctx.enter_context(tc.tile_pool(name="dram", bufs=1, space="DRAM"))

        # ---------------- constants into SBUF ----------------
        w0_sb = const.tile([3, W], F32R, name="w0_sb")
        nc.sync.dma_start(w0_sb[:], w0_d[:])
        w0r_sb = const.tile([128, 2, 3], F32, name="w0r_sb")
        nc.sync.dma_start(w0r_sb[:], w0r_d[:])
        w0q_sb = const.tile([128, 2, 2], F32, name="w0q_sb")
        nc.sync.dma_start(w0q_sb[:], w0q_d[:])
        w_sb = {}
        for l in range(1, DEPTH):
            t = const.tile([128, 2, W], F32R, name=f"w{l}_sb", tag=f"w{l}_sb")
            nc.sync.dma_start(t[:], w_d[l].rearrange("(k p) o -> p k o", p=128))
            w_sb[l] = t
        bc_sb = {}
        for l in range(DEPTH):
            t = const.tile([128, 2], F32, name=f"b{l}_sb", tag=f"b{l}_sb")
            nc.sync.dma_start(t[:], bc_d[l][:])
            bc_sb[l] = t
        w6s_sb = const.tile([128, 2, 5, 8], F32R, name="w6s_sb")
        nc.sync.dma_start(w6s_sb[:], w6s_d[:])
        b6b_sb = const.tile([128, 1], F32, name="b6b_sb")
        nc.sync.dma_start(b6b_sb[:], b6b_d[:])

        hrows = dram.tile([8, S], F32, name="hrows")

        # ---------------- per-layer bodies ----------------
        def mm_group(pt, m, wl, rhs_tiles):
            """Accumulate sum_k W[k,m]^T @ rhs[k] for each rhs into pt[:,m,:]."""
            n = 2 * len(rhs_tiles)
            i = 0
            for k in range(2):
                lhs = wl[:, k, m * 128:(m + 1) * 128]
                for t in rhs_tiles:
                    nc.tensor.matmul(
                        pt[:, m, :], lhs, t[:, k, :],
                        start=(i == 0), stop=(i == n - 1),
                    )
                    i += 1

        def nonlin(pa, bl):
            """tanh + derivative factors from the value pre-activation."""
            z = sb.tile([128, 2, BLK], F32R, name="st_z", tag="st_z")
            for m in range(2):
                nc.scalar.activation(z[:, m, :], pa[:, m, :], AF.Tanh,
                                     bias=bl[:, m:m + 1])
            z2 = tmp.tile([128, 2, BLK], F32, name="t_z2", tag="t_z2")
            nc.scalar.activation(z2[:], z[:], AF.Square)
            t1 = tmp.tile([128, 2, BLK], F32, name="t_t1", tag="t_t1")
            nc.vector.tensor_scalar(t1[:], z2[:], -1.0, 1.0, OP.mult, OP.add)
            t2 = tmp.tile([128, 2, BLK], F32, name="t_t2", tag="t_t2")
            nc.vector.scalar_tensor_tensor(t2[:], z[:], -2.0, t1[:],
                                           OP.mult, OP.mult)
            return z, t1, t2

        def layer0(blk):
            xb = xin.tile([3, BLK], F32R, name="xb", tag="xb")
            nc.sync.dma_start(xb[:], xyt_t[:, blk * BLK:(blk + 1) * BLK])
            pa = ps.tile([128, 2, BLK], F32, name="pa0", tag="ps")
            for m in range(2):
                nc.tensor.matmul(pa[:, m, :],
                                 w0_sb[:, m * 128:(m + 1) * 128],
                                 xb[:], start=True, stop=True)
            z, t1, t2 = nonlin(pa, bc_sb[0])
            st = {"z": z, "u1x": None, "u1y": None}
            for name, src, col in (("gx", w0r_sb, 0), ("gy", w0r_sb, 1),
                                   ("gt", w0r_sb, 2), ("u2x", w0q_sb, 0),
                                   ("u2y", w0q_sb, 1)):
                base = t2 if name.startswith("u2") else t1
                t = sb.tile([128, 2, BLK], F32R, name=f"st_{name}",
                            tag=f"st_{name}")
                for m in range(2):
                    nc.vector.tensor_scalar(t[:, m, :], base[:, m, :],
                                            src[:, m, col:col + 1], None,
                                            OP.mult)
                st[name] = t
            return st

        def hidden(l, old):
            wl = w_sb[l]
            pa = ps.tile([128, 2, BLK], F32, name="pa", tag="ps")
            for m in range(2):
                mm_group(pa, m, wl, [old["z"]])
            z, t1, t2 = nonlin(pa, bc_sb[l])
            st = {"z": z}

            for g, u1, u2 in (("gx", "u1x", "u2x"), ("gy", "u1y", "u2y")):
                pg = ps.tile([128, 2, BLK], F32, name=f"p{g}", tag="ps")
                for m in range(2):
                    mm_group(pg, m, wl, [old[g]])
                q = tmp.tile([128, 2, BLK], F32, name=f"t_q{g}",
                             tag=f"t_q{g}")
                nc.scalar.activation(q[:], pg[:], AF.Square)
                gn = sb.tile([128, 2, BLK], F32R, name=f"st_{g}",
                             tag=f"st_{g}")
                nc.vector.tensor_mul(gn[:], t1[:], pg[:])
                st[g] = gn
                un2 = sb.tile([128, 2, BLK], F32R, name=f"st_{u2}",
                              tag=f"st_{u2}")
                nc.vector.tensor_mul(un2[:], t2[:], q[:])
                st[u2] = un2
                psx = ps.tile([128, 2, BLK], F32, name=f"ps{u1}", tag="ps")
                rhs = [old[u1], old[u2]] if old[u1] is not None else [old[u2]]
                for m in range(2):
                    mm_group(psx, m, wl, rhs)
                un1 = sb.tile([128, 2, BLK], F32R, name=f"st_{u1}",
                              tag=f"st_{u1}")
                nc.vector.tensor_mul(un1[:], t1[:], psx[:])
                st[u1] = un1

            pgt = ps.tile([128, 2, BLK], F32, name="pgt", tag="ps")
            for m in range(2):
                mm_group(pgt, m, wl, [old["gt"]])
            gt = sb.tile([128, 2, BLK], F32R, name="st_gt", tag="st_gt")
            nc.vector.tensor_mul(gt[:], t1[:], pgt[:])
            st["gt"] = gt
            return st

        def final(blk, old):
            p6 = psf.tile([8, BLK], F32, name="p6", tag="p6")
            rhs_rows = [("z", 0), ("gx", 1), ("gy", 2), ("gt", 3),
                        ("u1x", 4), ("u2x", 4), ("u1y", 4), ("u2y", 4)]
            i = 0
            for k in range(2):
                for sname, row in rhs_rows:
                    nc.tensor.matmul(p6[:, :], w6s_sb[:, k, row, :],
                                     old[sname][:, k, :],
                                     start=(i == 0), stop=(i == 15))
                    i += 1
            hb6 = xin.tile([8, BLK], F32, name="hb6", tag="hb6")
            nc.scalar.activation(hb6[:], p6[:], AF.Copy)
            nc.sync.dma_start(hrows[:, blk * BLK:(blk + 1) * BLK], hb6[:])

        # ---------------- main point-block loop ----------------
        for blk in range(NBLK):
            st = layer0(blk)
            for l in range(1, DEPTH):
                st = hidden(l, st)
            final(blk, st)

        # ---------------- residual in packed layout ----------------
        pk = []
        for r in range(5):
            t = rp.tile([128, PK], F32, name=f"pk{r}", tag=f"pk{r}")
            nc.sync.dma_start(t[:], hrows[r].rearrange("(p c) -> p c", p=128))
            pk.append(t)
        h_, hx_, hy_, ht_, hss_ = pk
        xp = []
        for d in range(3):
            t = rp.tile([128, PK], F32, name=f"xp{d}", tag=f"xp{d}")
            nc.sync.dma_start(t[:], xyt_pack[d])
            xp.append(t)

        sinx = rp.tile([128, PK], F32, name="sinx")
        nc.scalar.activation(sinx[:], xp[0][:], AF.Sin, scale=PI)
        siny = rp.tile([128, PK], F32, name="siny")
        nc.scalar.activation(siny[:], xp[1][:], AF.Sin, scale=PI)
        expt = rp.tile([128, PK], F32, name="expt")
        nc.scalar.activation(expt[:], xp[2][:], AF.Exp, scale=-1.0)

        nf = rp.tile([128, PK], F32, name="nf")
        nc.vector.scalar_tensor_tensor(nf[:], sinx[:], -1.0, siny[:],
                                       OP.mult, OP.mult)
        nf2 = rp.tile([128, PK], F32, name="nf2")
        nc.vector.tensor_mul(nf2[:], nf[:], expt[:])
        # hb = (h + b6) * (-K)
        hb = rp.tile([128, PK], F32, name="hb")
        nc.vector.tensor_scalar(hb[:], h_[:], b6b_sb[:, 0:1], -KC,
                                OP.add, OP.mult)
        p1 = rp.tile([128, PK], F32, name="p1")
        nc.vector.tensor_mul(p1[:], hb[:], hss_[:])
        p2 = rp.tile([128, PK], F32, name="p2")
        nc.vector.scalar_tensor_tensor(p2[:], hx_[:], -KC, hx_[:],
                                       OP.mult, OP.mult)
        p3 = rp.tile([128, PK], F32, name="p3")
        nc.vector.scalar_tensor_tensor(p3[:], hy_[:], -KC, hy_[:],
                                       OP.mult, OP.mult)
        a1 = rp.tile([128, PK], F32, name="a1")
        nc.vector.scalar_tensor_tensor(a1[:], ht_[:], MU, p1[:],
                                       OP.mult, OP.add)
        a2 = rp.tile([128, PK], F32, name="a2")
        nc.vector.tensor_add(a2[:], a1[:], p2[:])
        a3 = rp.tile([128, PK], F32, name="a3")
        nc.vector.tensor_add(a3[:], a2[:], p3[:])
        rt = rp.tile([128, PK], F32, name="rt")
        nc.vector.tensor_add(rt[:], a3[:], nf2[:])
        nc.sync.dma_start(res_d[:], rt[:])

    nc.finalize()
    return nc


_NC_CACHE = None


def _get_nc():
    global _NC_CACHE
    if _NC_CACHE is None:
        _NC_CACHE = build_nc()
    return _NC_CACHE


def _shared_inputs(inputs):
    f = np.float32
    w0 = np.ascontiguousarray(inputs["w0"], dtype=f)          # (3,256)
    w6 = np.ascontiguousarray(inputs["w6"], dtype=f)          # (256,1)
    shared = {"w0": w0}
    for l in range(1, DEPTH):
        shared[f"w{l}"] = np.ascontiguousarray(inputs[f"w{l}"], dtype=f)
    for l in range(DEPTH):
        b = np.asarray(inputs[f"b{l}"], dtype=f)               # (256,)
        shared[f"b{l}c"] = np.ascontiguousarray(b.reshape(2, 128).T)
    w0r = np.ascontiguousarray(
        w0.T.reshape(2, 128, 3).transpose(1, 0, 2))            # (128,2,3)
    shared["w0r"] = w0r
    shared["w0q"] = np.ascontiguousarray(w0r[..., 0:2] ** 2)   # (128,2,2)
    w6s = np.zeros((128, 2, 5, 8), dtype=f)
    for k in range(2):
        for r in range(5):
            w6s[:, k, r, r] = w6[k * 128:(k + 1) * 128, 0]
    shared["w6s"] = w6s
    b6 = np.asarray(inputs["b6"], dtype=f)
    shared["b6b"] = np.full((128, 1), b6[0], dtype=f)
    return shared


def kernel(**inputs):
    xyt = np.ascontiguousarray(np.asarray(inputs["xyt"], dtype=np.float32))
    shared = _shared_inputs(inputs)
    in_maps = []
    for c in range(NCORES):
        xt = np.ascontiguousarray(xyt[c * S:(c + 1) * S].T)    # (3,S)
        m = dict(shared)
        m["xyt_t"] = xt
        m["xyt_pack"] = np.ascontiguousarray(xt.reshape(3, 128, PK))
        in_maps.append(m)
    nc = _get_nc()
    out = run_bass_kernel_spmd(nc, in_maps, list(range(NCORES)))
    res = np.concatenate([r["res"].reshape(S) for r in out.results])
    return np.ascontiguousarray(res[:, None].astype(np.float32))


if __name__ == "__main__":
    rng = np.random.default_rng(0)
    dims = [3] + [W] * DEPTH + [1]
    ins = {"xyt": rng.random((NPTS, 3), dtype=np.float32)}
    for i in range(DEPTH + 1):
        ins[f"w{i}"] = (rng.standard_normal((dims[i], dims[i + 1]))
                        / np.sqrt(dims[i])).astype(np.float32)
        ins[f"b{i}"] = np.zeros((dims[i + 1],), dtype=np.float32)
    r = kernel(**ins)
    print("kernel ran, output", r.shape, r[:4, 0])
